# revision 69
# baseline (speedup 1.0000x reference)
"""Trainium2 Bass kernel for vq_codebook argmin (nn_GUMSampler).

Per pixel p (4M pixels), d2[v] = ||z_p - vertex_v||^2 over 16 vertices in
R^15; outputs argmin index (int32) and min distance (f32). Pixels sharded
8 ways across cores (contiguous blocks); vertices replicated; no
communication.

Math: d2 = zz - 2<V_v, z> + vv.  zz = ||z||^2 is computed on the HOST and
shipped as a 16th channel with matmul weight 1.0 (zz is common to all 16
candidates of a pixel, so its precision never affects the argmin); vv is
added as a per-partition bias during the ACT PSUM->SBUF copy.

Per-core pipeline (64 iters x 8192 pixels, EFF=1024 pixels/group-iter):
  - z pre-packed host-side into per-2-iter SBUF-ready blocks [128, 2048]:
    row 16g+c = channel c of pixel-group g (c<15), row 16g+15 = zz,
    split into bf16 hi/lo halves (zh, zl).  One contiguous dma per tensor
    per 2 iters (4-deep prefetch).
  - PSUM m2 [128=16g+v, 1024] via 3 bf16 matmuls per 512-col half:
    Wh@zh + Wh@zl + Wl@zh where -2V = Wh + Wl (bf16 splits, host-side).
    The dropped Wl@zl term is ~1e-5 -> exact-f32 quality at bf16 matmul
    speed (1 cyc/row vs 4 for fp32; f32r single-pass crashes neuronxcc).
  - ACT Identity copy PSUM->SBUF adds vv[v] per row (bias AP).
  - DVE packs the vertex id into the 4 low mantissa bits:
      packed = (bits(d2) & ~15) | v   (d2 >= ~1, 15 ulp ~ 1e-5 rel;
    f32 min then yields min-d2 AND argmin with first-index tie-break).
  - PE transposes each 128x128 block (pixels -> psum partitions, 16g+v
    along free); ONE DVE tensor_reduce(min) per half with AP [p,g,m,v:16]
    replaces the old partition min-tree.  (A real min-tree level on
    partitions is illegal: walrus requires equal base partitions for
    SBUF/SBUF tensor_tensor.)  The transposes for iteration i are emitted
    two iterations behind the matmuls (software pipelining) so they never
    head-block the in-order PE queue while waiting on the pack.
  - Reduced [128, 32] slabs scatter into S [128, 1024] at s = g*128 + b
    (b = 128-pixel-block index within a 16-iter epoch).  PE
    back-transposes S per group into T2 halves; ACT scatters them into a
    2-epoch staging D [128, 2048] so each group owns 256 contiguous
    columns; idx = packed & 15, dmin = sqrt(packed); one fully-contiguous
    128KB store per group per 32 iters via SWDGE (Pool) so store waits
    never head-block the SP queue that feeds z prefetches (final epoch:
    stores spread across SP/ACT/Pool to cut the drain tail).
"""

import sys

sys.path.insert(0, "/opt/trn_rl_repo")

from contextlib import ExitStack

import numpy as np

import concourse.bacc as bacc
import concourse.tile as tile
from concourse import mybir
from concourse.bass_utils import run_bass_kernel_spmd

F32 = mybir.dt.float32
BF16 = mybir.dt.bfloat16
I32 = mybir.dt.int32

K = 16          # vertices
C = 15          # channels (K-1)
G = 8           # pixel groups
EFF = 1024      # pixels per iteration per group
N_CORES = 8
LX = LY = 2048
N_TOTAL = LX * LY
N_LOC = N_TOTAL // N_CORES          # 524288 pixels per core
N_ITERS = N_LOC // (G * EFF)        # 64

MM_MODE = "bf16x2"  # "bf16x2" | "f32"  (f32r crashes neuronxcc on HW)
GPSIMD_COMPUTE = False  # use Pool engine for part of L1-min / pack

_CACHE = {}


def build_nc(n_iters=N_ITERS, mm_mode=None):
    if mm_mode is None:
        mm_mode = MM_MODE
    assert n_iters % 32 == 0
    n_loc = n_iters * G * EFF
    nc = bacc.Bacc("TRN2", target_bir_lowering=False, debug=False)

    n2 = n_iters // 2
    if mm_mode == "bf16x2":
        zh_d = nc.dram_tensor("zh", [n2, 128, 2 * EFF], BF16, kind="ExternalInput")
        zl_d = nc.dram_tensor("zl", [n2, 128, 2 * EFF], BF16, kind="ExternalInput")
        wh_d = nc.dram_tensor("wh", [128, 128], BF16, kind="ExternalInput")
        wl_d = nc.dram_tensor("wl", [128, 128], BF16, kind="ExternalInput")
    else:
        z_d = nc.dram_tensor("z", [n2, 128, 2 * EFF], F32, kind="ExternalInput")
        w1z_d = nc.dram_tensor("w1z", [128, 128], F32, kind="ExternalInput")
    vvec_d = nc.dram_tensor("vvec", [128, 1], I32, kind="ExternalInput")
    vvb_d = nc.dram_tensor("vvb", [128, 1], F32, kind="ExternalInput")
    ident_d = nc.dram_tensor("ident", [128, 128], F32, kind="ExternalInput")
    idx_d = nc.dram_tensor("idx", [n_loc], I32, kind="ExternalOutput")
    dmin_d = nc.dram_tensor("dmin", [n_loc], F32, kind="ExternalOutput")

    AND_MASK = -16  # 0xFFFFFFF0
    MIN = mybir.AluOpType.min
    IDENT = mybir.ActivationFunctionType.Identity

    with tile.TileContext(nc) as tc, ExitStack() as ctx:
        cpool = ctx.enter_context(tc.tile_pool(name="consts", bufs=1))
        if mm_mode == "bf16x2":
            wh_s = cpool.tile([128, 128], BF16)
            wl_s = cpool.tile([128, 128], BF16)
            nc.sync.dma_start(wh_s[:], wh_d[:])
        else:
            w1z_s = cpool.tile([128, 128], F32)
            nc.sync.dma_start(w1z_s[:], w1z_d[:])
        vvec_s = cpool.tile([128, 1], I32)
        vvb_s = cpool.tile([128, 1], F32)
        ident_s = cpool.tile([128, 128], F32)
        # aux consts ride SWDGE so they reach the dma device ~1us late,
        # letting the first z block jump ahead; vvb/vvec are needed by
        # ACT/DVE ~2.5us in, ident by the first transposes ~4us in
        nc.scalar.dma_start(vvb_s[:], vvb_d[:])
        nc.gpsimd.dma_start(vvec_s[:], vvec_d[:])
        nc.gpsimd.dma_start(ident_s[:], ident_d[:])

        if mm_mode == "bf16x2":
            zhpool = ctx.enter_context(tc.tile_pool(name="zh", bufs=4))
            zlpool = ctx.enter_context(tc.tile_pool(name="zl", bufs=4))
        else:
            zpool = ctx.enter_context(tc.tile_pool(name="zt", bufs=3))
        pspool = ctx.enter_context(tc.tile_pool(name="ps", bufs=2, space="PSUM"))
        psbpool = ctx.enter_context(tc.tile_pool(name="psb", bufs=3))
        pkpool = ctx.enter_context(tc.tile_pool(name="pk", bufs=3))
        tpool = ctx.enter_context(tc.tile_pool(name="tr", bufs=2, space="PSUM"))
        spool = ctx.enter_context(tc.tile_pool(name="smin", bufs=2))
        dpool = ctx.enter_context(tc.tile_pool(name="dpk", bufs=2))
        dxpool = ctx.enter_context(tc.tile_pool(name="dix", bufs=2))
        ddpool = ctx.enter_context(tc.tile_pool(name="ddm", bufs=2))

        # store views: P = g*gblk + e*32768 + x,
        #   x = (nl*128 + 8*(i%16) + 4*h + m)*128 + p, nl = (i%32)//16
        ixv = idx_d[:].rearrange(
            "(g e x) -> e g x", g=G, e=n_iters // 32, x=32768
        )
        dmv = dmin_d[:].rearrange(
            "(g e x) -> e g x", g=G, e=n_iters // 32, x=32768
        )

        state = {"S": None, "D": None, "pending": []}

        def emit_head(i):
            """dma/square/matmuls/copy/pack for iteration i."""
            if i % 2 == 0:
                if mm_mode == "bf16x2":
                    zh = zhpool.tile([128, 2 * EFF], BF16)
                    zl = zlpool.tile([128, 2 * EFF], BF16)
                    if i == 0:
                        # split the first loads so matmul 0 starts early;
                        # wl is only needed by the 3rd matmul
                        nc.sync.dma_start(zh[:, 0:512], zh_d[0, :, 0:512])
                        nc.sync.dma_start(zl[:, 0:512], zl_d[0, :, 0:512])
                        nc.sync.dma_start(wl_s[:], wl_d[:])
                        nc.sync.dma_start(zh[:, 512:], zh_d[0, :, 512:])
                        nc.sync.dma_start(zl[:, 512:], zl_d[0, :, 512:])
                    else:
                        nc.sync.dma_start(zh[:], zh_d[i // 2])
                        nc.sync.dma_start(zl[:], zl_d[i // 2])
                    state["zh"], state["zl"] = zh, zl
                else:
                    z2 = zpool.tile([128, 2 * EFF], F32)
                    nc.sync.dma_start(z2[:], z_d[i // 2])
                    state["z2"] = z2

            col = (i % 2) * EFF
            ps = pspool.tile([128, EFF], F32, space="PSUM")
            for h in (0, 1):
                sl = slice(512 * h, 512 * h + 512)
                a = col + 512 * h
                if mm_mode == "bf16x2":
                    zh, zl = state["zh"], state["zl"]
                    nc.tensor.matmul(
                        ps[:, sl], wh_s[:], zh[:, a : a + 512],
                        start=True, stop=False,
                    )
                    nc.tensor.matmul(
                        ps[:, sl], wh_s[:], zl[:, a : a + 512],
                        start=False, stop=False,
                    )
                    nc.tensor.matmul(
                        ps[:, sl], wl_s[:], zh[:, a : a + 512],
                        start=False, stop=True,
                    )
                else:
                    nc.tensor.matmul(
                        ps[:, sl], w1z_s[:], state["z2"][:, a : a + 512],
                        start=True, stop=True,
                    )

            # PSUM -> SBUF with per-row bias vv[v] (Identity activation)
            psb = psbpool.tile([128, EFF], F32)
            nc.scalar.activation(psb[:], ps[:], IDENT, bias=vvb_s[:])

            pk = pkpool.tile([128, EFF], F32)
            if GPSIMD_COMPUTE:
                nc.vector.tensor_scalar(
                    pk[:, 0:768].bitcast(I32), psb[:, 0:768].bitcast(I32),
                    AND_MASK, vvec_s[:],
                    op0=mybir.AluOpType.bitwise_and,
                    op1=mybir.AluOpType.bitwise_or,
                )
                nc.gpsimd.tensor_scalar(
                    pk[:, 768:EFF].bitcast(I32), psb[:, 768:EFF].bitcast(I32),
                    AND_MASK, vvec_s[:],
                    op0=mybir.AluOpType.bitwise_and,
                    op1=mybir.AluOpType.bitwise_or,
                )
            else:
                nc.vector.tensor_scalar(
                    pk[:].bitcast(I32), psb[:].bitcast(I32), AND_MASK,
                    vvec_s[:],
                    op0=mybir.AluOpType.bitwise_and,
                    op1=mybir.AluOpType.bitwise_or,
                )
            return pk

        def emit_tail(i, pk):
            """min-tree level, transposes, reduce for iter i, epoch stores."""
            if i % 16 == 0:
                state["S"] = spool.tile([128, 1024], F32, name="S")
            S = state["S"]

            for h in (0, 1):
                T = tpool.tile([128, 512], F32, space="PSUM")
                for m in range(4):
                    nc.tensor.transpose(
                        T[:, 128 * m : 128 * m + 128],
                        pk[:, 512 * h + 128 * m : 512 * h + 128 * m + 128],
                        ident_s[:],
                    )
                rin = T[:].rearrange("p (m g v) -> p g m v", m=4, g=G, v=K)
                a = (i % 16) * 8 + 4 * h
                rout = S[:].rearrange("p (g b) -> p g b", g=G, b=128)[
                    :, :, a : a + 4
                ]
                nc.vector.tensor_reduce(
                    rout, rin, axis=mybir.AxisListType.X, op=MIN
                )

            if i % 16 == 15:
                nl = (i % 32) // 16
                if nl == 0:
                    state["D"] = dpool.tile([128, 2048], F32, name="D")
                D = state["D"]
                dv = D[:].rearrange("p (g nl b) -> p g nl b", g=G, nl=2, b=128)
                # back-transpose S per group through tpool halves; scatter
                # epoch-half nl: T2 block g -> D cols 256g+128nl
                for half in (0, 1):
                    T2 = tpool.tile([128, 512], F32, space="PSUM", name="T2")
                    for gg in range(4):
                        g = 4 * half + gg
                        nc.tensor.transpose(
                            T2[:, 128 * gg : 128 * gg + 128],
                            S[:, 128 * g : 128 * g + 128],
                            ident_s[:],
                        )
                    dvh = dv[:, 4 * half : 4 * half + 4, nl]
                    t2v = T2[:].rearrange("p (g b) -> p g b", g=4, b=128)
                    if i == n_iters - 1 and half == 1:
                        # final drain: run the 2nd scatter on DVE so it
                        # overlaps the ACT one
                        nc.vector.tensor_copy(dvh, t2v)
                    else:
                        nc.scalar.copy(dvh, t2v)

            fin_e = (n_iters - 1) // 32
            # early-drain the final epoch's first half: its staging data is
            # complete at i = 32*fin_e + 15 and would otherwise sit idle
            # until the end-of-kernel drain
            early = i == 32 * fin_e + 15 and n_iters >= 32
            if i % 32 == 31 or early:
                e = i // 32
                final = i == n_iters - 1
                D = state["D"]
                if early:
                    nls = (0,)
                elif e == fin_e:
                    nls = (1,)          # nl=0 already early-drained
                else:
                    nls = (0, 1)
                for nn in nls:
                    dvv = D[:].rearrange(
                        "p (g nl b) -> p g nl b", g=G, nl=2, b=128
                    )[:, :, nn]
                    Dx = dxpool.tile([128, 1024], I32)
                    nc.vector.tensor_scalar(
                        Dx[:].rearrange("p (g b) -> p g b", g=G, b=128),
                        dvv.bitcast(I32), 15, None,
                        op0=mybir.AluOpType.bitwise_and,
                    )
                    Dd = ddpool.tile([128, 1024], F32)
                    nc.scalar.sqrt(
                        Dd[:].rearrange("p (g b) -> p g b", g=G, b=128), dvv
                    )
                    for g in range(G):
                        sl = slice(128 * g, 128 * g + 128)
                        dsx = ixv[e, g].rearrange(
                            "(ll a p) -> ll a p", ll=2, a=128, p=128
                        )[nn]
                        dsd = dmv[e, g].rearrange(
                            "(ll a p) -> ll a p", ll=2, a=128, p=128
                        )[nn]
                        if final:
                            # nothing queues behind: spread across engines
                            qx = (nc.sync, nc.scalar, nc.gpsimd, nc.sync,
                                  nc.scalar, nc.gpsimd, nc.sync, nc.scalar)[g]
                            qd = (nc.gpsimd, nc.sync, nc.scalar, nc.gpsimd,
                                  nc.sync, nc.scalar, nc.gpsimd, nc.sync)[g]
                            qx.dma_start(dsx, Dx[:, sl])
                            qd.dma_start(dsd, Dd[:, sl])
                        else:
                            # defer: spread over following iterations so the
                            # dma-device burst never starves z prefetches
                            state["pending"].append((dsx, Dx[:, sl]))
                            state["pending"].append((dsd, Dd[:, sl]))

        # software-pipelined: head(i) runs two iterations ahead of tail(i-2)
        pks = [emit_head(0), emit_head(1)]
        for i in range(2, n_iters):
            pks.append(emit_head(i))
            emit_tail(i - 2, pks.pop(0))
            if state["pending"]:
                dst, srcp = state["pending"].pop(0)
                nc.gpsimd.dma_start(dst, srcp)
        emit_tail(n_iters - 2, pks.pop(0))
        emit_tail(n_iters - 1, pks.pop(0))
        for dst, srcp in state["pending"]:
            nc.gpsimd.dma_start(dst, srcp)

    nc.compile()
    return nc


def _weights(vertices, mm_mode=None):
    if mm_mode is None:
        mm_mode = MM_MODE
    V = np.asarray(vertices, dtype=np.float32)          # (16, 15)
    vv = (V.astype(np.float64) ** 2).sum(1).astype(np.float32)
    w1z = np.zeros((128, 128), dtype=np.float32)
    for g in range(G):
        for v in range(K):
            w1z[16 * g : 16 * g + C, 16 * g + v] = -2.0 * V[v]
            w1z[16 * g + C, 16 * g + v] = 1.0      # zz row
    vvec = (np.arange(128, dtype=np.int32) & 15).reshape(128, 1)
    vvb = vv[np.arange(128) & 15].astype(np.float32).reshape(128, 1)
    ident = np.eye(128, dtype=np.float32)
    out = {"vvec": vvec, "vvb": vvb, "ident": ident}
    bf = mybir.dt.np(BF16)
    if mm_mode == "bf16x2":
        wh = w1z.astype(bf)
        wl = (w1z - wh.astype(np.float32)).astype(bf)
        out["wh"] = wh
        out["wl"] = wl
    else:
        out["w1z"] = w1z
    return out


def _pack_z(z_fl_core, mm_mode=None):
    """[15, n_loc] -> per-2-iter SBUF-ready [n2, 128, 2048] blocks.

    Row 16g+c = channel c (c<15), row 16g+15 = zz = sum_c z_c^2.
    Returns {"z": f32} or {"zh", "zl"} bf16 hi/lo splits.
    """
    if mm_mode is None:
        mm_mode = MM_MODE
    z = np.asarray(z_fl_core, dtype=np.float32)
    n_loc = z.shape[1]
    n2 = n_loc // (G * 2 * EFF)
    zz = np.einsum("ij,ij->j", z, z, dtype=np.float64).astype(np.float32)
    zaug = np.empty((16, G, n2, 2 * EFF), dtype=np.float32)
    zaug[0:C] = z.reshape(C, G, n2, 2 * EFF)
    zaug[C] = zz.reshape(G, n2, 2 * EFF)
    # -> [n2, (g, c16), y]
    zaug = np.ascontiguousarray(
        zaug.transpose(2, 1, 0, 3).reshape(n2, 128, 2 * EFF)
    )
    if mm_mode != "bf16x2":
        return {"z": zaug}
    bf = mybir.dt.np(BF16)
    zh = zaug.astype(bf)
    zl = (zaug - zh.astype(np.float32)).astype(bf)
    return {"zh": zh, "zl": zl}


def make_in_map(z_fl, vertices, mm_mode=None):
    m = _weights(vertices, mm_mode)
    m.update(_pack_z(z_fl, mm_mode))
    return m


def kernel(z, vertices):
    z = np.ascontiguousarray(np.asarray(z, dtype=np.float32))
    lx, ly = z.shape[1], z.shape[2]
    n = lx * ly
    z_fl = z.reshape(C, n)
    n_loc = n // N_CORES

    if "nc" not in _CACHE:
        _CACHE["nc"] = build_nc()
    nc = _CACHE["nc"]

    w = _weights(vertices)
    in_maps = []
    for c in range(N_CORES):
        m = dict(w)
        m.update(_pack_z(z_fl[:, c * n_loc : (c + 1) * n_loc]))
        in_maps.append(m)
    res = run_bass_kernel_spmd(nc, in_maps, list(range(N_CORES)))
    X = np.concatenate([res.results[c]["idx"] for c in range(N_CORES)])
    dmin = np.concatenate([res.results[c]["dmin"] for c in range(N_CORES)])
    return X.reshape(lx, ly), dmin.reshape(lx, ly)


if __name__ == "__main__":
    print("smoke build only")
    build_nc(32)
    print("ok")
